# revision 25
# baseline (speedup 1.0000x reference)
"""GroupQuantLinear int4 dequant + linear on 8 Trainium2 NeuronCores.

y = x @ W^T,  W = dequant(w_packed)*w_scale + w_bias  (group size 64)

Column-parallel sharding: 1536 output rows per core, x replicated.
Per core the contraction K=8192 is split into 64 k-tiles of 128
partitions (partition == quant group, k == position in group), plus one
extra k-tile that multiplies per-group x sums against the bias rows so
the bias term rides the same PSUM accumulation. 12 o-tiles of 128 rows
run as 2 passes of 6 PSUM banks; out [128 o, 512 t] fp32 per bank.

Performance structure (measured on HW, perfetto-profiled):
  - Mixed-precision contraction: the first 50 k-tiles are bf16
    (u8 nibbles streamed from HBM, dequantized on the DVE with a single
    u8*bf16 multiply per k-tile); the last NF=14 k-tiles are fp8e4
    DoubleRow pairs (2 contraction slices per matmul = 2x PE rate),
    host-dequantized so they cost the same HBM bytes as nibbles and no
    DVE work. fp8 error scales as sqrt(NF/64)*3.7%; measured rel err
    1.75e-2 against the fp32 reference (gate 2e-2).
  - ~30 dummy matmuls on a zeroed tile right after the framework
    preamble keep the PE busy from t~7.5us so the HAM clock-gate is at
    8/8 (2.4 GHz) before the real matmuls start, and bias matmuls are
    ordered first so the PE never goes idle (idle >3.4us re-throttles
    to 1.2 GHz).
  - The early DMA window is latency-critical and bandwidth-starved
    (~150 GB/s effective): only the opening tensors (bias/scale halves,
    xsum, first x and nibble chunks) are issued up front; all other
    input streams (remaining x chunks, second bias/scale halves, fp8 x
    and weights) are released inside the loop, gated on dequant
    progress via tiny gate-copies, so the SDMA engines never round-rob
    the critical path against bulk traffic.
  - Weight nibbles stream on the sync HWDGE queue in ramped chunks with
    a small split head per chunk; x streams on the scalar HWDGE queue;
    outputs leave as bf16 (halves output bytes; host converts to f32).
  - PSUM evacuation: pass-0 on scalar copies + gpsimd DMAs mid-kernel;
    the final pass alternates vector/scalar copies and gpsimd/sync DMA
    queues to shorten the post-last-matmul tail.
"""
import os
import sys

for _p in ("/opt/trn_rl_repo",):
    if _p not in sys.path and os.path.isdir(_p):
        sys.path.insert(0, _p)

import numpy as np
import ml_dtypes

import concourse.bacc as bacc
import concourse.mybir as mybir
import concourse.tile as tile
from concourse import bass_utils

# ---- problem constants (hardcoded per contract) ----
B, S, IN_F, OUT_F = 4, 128, 8192, 12288
GS = 64                 # quant group size
NG = IN_F // GS         # 128 groups == partitions per k-tile
N_CORES = 8
O_CORE = OUT_F // N_CORES   # 1536
T = B * S                   # 512 tokens
N_OPASS = 2                 # PSUM-capacity passes over output tiles
N_WARM = 30                 # HAM-warmup dummy matmuls
NF = 14                     # trailing k-tiles computed in fp8 DoubleRow
NB16 = GS - NF              # leading bf16 k-tiles
NPAIR = NF // 2
NKB = 1 + NB16              # xsum + bf16 k-tiles in the bf16 x tensor

F8 = ml_dtypes.float8_e4m3  # TRN float8e4: e4m3, bias 7, max +-240


def host_prep_x(x):
    """x [B,S,I] fp32 -> (xtb [128, NKB, T] bf16, xt8 [128, NPAIR, 2, T] fp8).

    xtb[:,0] is the per-group x sum (bias k-tile); xtb[:,1+k] is group
    position k for k < NB16. xt8[:,p,i] is group position NB16 + 2p + i.
    """
    x2 = x.reshape(T, NG, GS)
    xtb = np.empty((NG, NKB, T), dtype=np.float32)
    xtb[:, 0] = x2.sum(axis=2, dtype=np.float64).T
    xtb[:, 1:] = x2.transpose(1, 2, 0)[:, :NB16]
    xt8 = np.ascontiguousarray(
        x2.transpose(1, 2, 0)[:, NB16:].reshape(NG, NPAIR, 2, T))
    return xtb.astype(ml_dtypes.bfloat16), xt8.astype(F8)


def host_prep_w(w_packed, w_scale, w_bias):
    """-> per-core (wn u8 nibbles for the bf16 k-tiles, w8 fp8 dequantized
    weights for the trailing NF k-tiles (paired for DoubleRow), sT bf16,
    bT bf16). The fp8 section is dequantized on the host: same bytes as
    nibbles in HBM, but zero on-chip DVE work."""
    p4 = w_packed.reshape(OUT_F, NG, 4, 4)
    nibs = np.stack([(p4 >> (4 * i)) & 0xF for i in range(4)], axis=-2)
    u = nibs.reshape(OUT_F, NG, GS).astype(np.uint8)        # [O, G, 64]
    OH = O_CORE // N_OPASS
    wns, w8s, sts, bts = [], [], [], []
    for c in range(N_CORES):
        sl = slice(c * O_CORE, (c + 1) * O_CORE)
        uc = u[sl].transpose(1, 2, 0)                        # [128, 64, Oc]
        st = np.ascontiguousarray(w_scale[sl, :, 0].T).astype(ml_dtypes.bfloat16)
        wn = np.empty((N_OPASS, NG, NB16, OH), dtype=np.uint8)
        w8 = np.empty((N_OPASS, NG, NPAIR, 2, OH), dtype=F8)
        wq = (uc[:, NB16:].astype(np.float32)
              * st.astype(np.float32)[:, None, :])           # [128, NF, Oc]
        for p in range(N_OPASS):
            wn[p] = uc[:, :NB16, p * OH:(p + 1) * OH]
            w8[p] = wq[:, :, p * OH:(p + 1) * OH].reshape(
                NG, NPAIR, 2, OH).astype(F8)
        wns.append(wn)
        w8s.append(w8)
        sts.append(st)
        bts.append(np.ascontiguousarray(w_bias[sl, :, 0].T)
                   .astype(ml_dtypes.bfloat16))
    return wns, w8s, sts, bts


def build():
    """Build the per-core bass program (identical on all cores)."""
    NOJ = O_CORE // 128
    OPP = NOJ // N_OPASS
    OH = OPP * 128

    WCH0 = [1, 3, 4] + [8] * 5 + [2]      # pass-0 nibble chunks (sum NB16)
    WCH1 = [8] * 6 + [2]                  # pass-1 nibble chunks (sum NB16)
    assert sum(WCH0) == NB16 and sum(WCH1) == NB16
    # bf16 x chunks. The first N_OPEN_X k-tiles ride the sync queue right
    # after the first weight chunk (in-queue order keeps them behind the
    # latency-critical tensors); the rest are released inside the pass-0
    # loop, gated on dequant progress, so the SDMA engines are not flooded
    # with x traffic while the opening tensors and nibble stream flow.
    N_OPEN_X = 4
    XCH_OPEN = [1, 1, 2]                  # upfront scalar chunks (sum N_OPEN_X)
    XCH = [2, 2] + [4] * 10 + [2]         # gated chunks (sum NB16 - N_OPEN_X)
    assert sum(XCH_OPEN) == N_OPEN_X and sum(XCH) == NB16 - N_OPEN_X

    nc = bacc.Bacc("TRN2", target_bir_lowering=False)
    xt_d = nc.dram_tensor("xt", [NG, NKB, T], mybir.dt.bfloat16,
                          kind="ExternalInput")
    x8_d = nc.dram_tensor("x8", [NG, NPAIR, 2, T], mybir.dt.float8e4,
                          kind="ExternalInput")
    wn_d = nc.dram_tensor("wn", [N_OPASS, NG, NB16, OH], mybir.dt.uint8,
                          kind="ExternalInput")
    w8_d = nc.dram_tensor("w8", [N_OPASS, NG, NPAIR, 2, OH],
                          mybir.dt.float8e4, kind="ExternalInput")
    st_d = nc.dram_tensor("st", [NG, O_CORE], mybir.dt.bfloat16, kind="ExternalInput")
    bt_d = nc.dram_tensor("bt", [NG, O_CORE], mybir.dt.bfloat16, kind="ExternalInput")
    yt_d = nc.dram_tensor("yt", [O_CORE, T], mybir.dt.bfloat16,
                          kind="ExternalOutput")

    with tile.TileContext(nc) as tc:
        with (
            tc.tile_pool(name="resident", bufs=1) as rpool,
            tc.tile_pool(name="nibs", bufs=3) as bpool,
            tc.tile_pool(name="wts", bufs=12) as wpool,
            tc.tile_pool(name="wts8", bufs=2) as w8pool,
            tc.tile_pool(name="evac", bufs=6) as opool,
            tc.tile_pool(name="psum", bufs=8, space="PSUM") as ppool,
        ):
            # --- PE prewarm: dummy matmuls on a zeroed tile so the HAM
            # clock-gate is already 8/8 when the first real matmul issues.
            warm_w = rpool.tile([128, 128], mybir.dt.bfloat16)
            nc.vector.memset(warm_w[:], 0)
            warm_ps = ppool.tile([128, T], mybir.dt.float32, tag="ps",
                                 name="warm")
            for _ in range(N_WARM):
                nc.tensor.matmul(warm_ps[:, :128], warm_w[:], warm_w[:],
                                 start=True, stop=True, skip_group_check=True)

            # --- opening DMAs, one stream per queue ---
            st_s = rpool.tile([NG, O_CORE], mybir.dt.bfloat16)
            bt_s = rpool.tile([NG, O_CORE], mybir.dt.bfloat16)
            x8_s = rpool.tile([NG, NPAIR, 2, T], mybir.dt.float8e4)
            xt_s = rpool.tile([NG, NKB, T], mybir.dt.bfloat16)
            nc.scalar.dma_start(bt_s[:, :OH], bt_d[:, :OH])
            nc.scalar.dma_start(st_s[:, :OH], st_d[:, :OH])
            nc.sync.dma_start(xt_s[:, 0:1, :], xt_d[:, 0:1, :])
            gate_s = rpool.tile([1, 2], mybir.dt.bfloat16)
            k0 = 1
            for ch in XCH_OPEN:
                nc.scalar.dma_start(xt_s[:, k0:k0 + ch, :],
                                    xt_d[:, k0:k0 + ch, :])
                k0 += ch
            # map: dequant k -> x chunks to release right after it (10 k-tile
            # lead over the first matmul that consumes the chunk)
            release_at = {}
            kx = N_OPEN_X                     # first k-tile of next chunk
            for ch in XCH:
                release_at.setdefault(max(0, kx - 10), []).append((kx + 1, ch))
                kx += ch
            X8_GATE_K = 28                    # release x8 after this dequant

            for p in range(N_OPASS):
                oo = p * OH
                psums = [ppool.tile([128, T], mybir.dt.float32, tag="ps",
                                    name=f"ps_{p}_{j}")
                         for j in range(OPP)]
                # bias k-tile first: needs only xsum (xt idx 0) + bt
                for j in range(OPP):
                    nc.tensor.matmul(
                        psums[j][:],
                        bt_s[:, oo + j * 128: oo + (j + 1) * 128],
                        xt_s[:, 0, :],
                        start=True, stop=False)
                k0 = 0
                for ch in (WCH0 if p == 0 else WCH1):
                    nt = bpool.tile([NG, ch, OH], mybir.dt.uint8, tag="nib",
                                    name=f"nib_{p}_{k0}")
                    if ch >= 4:
                        # split the chunk DMA so the first dequants can
                        # start before the whole chunk has landed
                        nc.sync.dma_start(nt[:, :2, :],
                                          wn_d[p, :, k0:k0 + 2, :])
                        nc.sync.dma_start(nt[:, 2:, :],
                                          wn_d[p, :, k0 + 2:k0 + ch, :])
                    else:
                        nc.sync.dma_start(nt[:], wn_d[p, :, k0:k0 + ch, :])
                    for kk in range(ch):
                        k = k0 + kk
                        if True:
                            wt = wpool.tile([NG, OH], mybir.dt.bfloat16,
                                            tag="wt")
                            nc.vector.tensor_mul(wt[:], nt[:, kk, :],
                                                 st_s[:, oo:oo + OH])
                            if p == 0:
                                # release paced x/x8 DMAs tied to dequant
                                # progress via tiny gate copies (FIFO on the
                                # issuing engine orders the dma after them)
                                for (xs, ch2) in release_at.get(k, ()):
                                    nc.scalar.copy(gate_s[:], wt[:1, :2])
                                    nc.scalar.dma_start(
                                        xt_s[:, xs:xs + ch2, :],
                                        xt_d[:, xs:xs + ch2, :])
                                if k == 16:
                                    nc.scalar.copy(gate_s[:], wt[:1, :2])
                                    nc.scalar.dma_start(bt_s[:, OH:],
                                                        bt_d[:, OH:])
                                elif k == 20:
                                    nc.scalar.copy(gate_s[:], wt[:1, :2])
                                    nc.scalar.dma_start(st_s[:, OH:],
                                                        st_d[:, OH:])
                                elif k == X8_GATE_K:
                                    nc.scalar.copy(gate_s[:], wt[:1, :2])
                                    nc.scalar.dma_start(x8_s[:], x8_d[:])
                            for j in range(OPP):
                                nc.tensor.matmul(
                                    psums[j][:],
                                    wt[:, j * 128:(j + 1) * 128],
                                    xt_s[:, k + 1, :],
                                    start=False, stop=False)
                    k0 += ch
                # trailing NF k-tiles: host-dequantized fp8 DoubleRow pairs
                w8s = w8pool.tile([NG, NPAIR, 2, OH], mybir.dt.float8e4,
                                  tag="w8")
                nc.sync.dma_start(w8s[:, :2], w8_d[p, :, :2])
                nc.sync.dma_start(w8s[:, 2:], w8_d[p, :, 2:])
                for pr in range(NPAIR):
                    for j in range(OPP):
                        nc.tensor.matmul(
                            psums[j][:],
                            w8s[:, pr, :, j * 128:(j + 1) * 128],
                            x8_s[:, pr, :, :],
                            start=False, stop=(pr == NPAIR - 1),
                            perf_mode=mybir.MatmulPerfMode.DoubleRow)
                if p == 0:
                    # mid-kernel evacuation: scalar copies (ACT can read
                    # PSUM; it is idle here), DMAs on the idle gpsimd queue
                    for j in range(OPP):
                        ot = opool.tile([128, T], mybir.dt.bfloat16, tag="ot")
                        nc.scalar.copy(ot[:], psums[j][:])
                        nc.gpsimd.dma_start(
                            yt_d[oo + j * 128: oo + (j + 1) * 128, :], ot[:])
                else:
                    # final evacuation: alternate vector/scalar copies and
                    # gpsimd/sync DMA queues so the post-last-matmul tail
                    # is short (gpsimd cannot read PSUM, so no copies there)
                    for j in range(OPP):
                        ot = opool.tile([128, T], mybir.dt.bfloat16, tag="ot")
                        if j % 2 == 0:
                            nc.vector.tensor_copy(ot[:], psums[j][:])
                            nc.gpsimd.dma_start(
                                yt_d[oo + j * 128: oo + (j + 1) * 128, :], ot[:])
                        else:
                            nc.scalar.copy(ot[:], psums[j][:])
                            nc.sync.dma_start(
                                yt_d[oo + j * 128: oo + (j + 1) * 128, :], ot[:])

    nc.compile()
    return nc


_NC_CACHE = None


def get_nc():
    global _NC_CACHE
    if _NC_CACHE is None:
        _NC_CACHE = build()
    return _NC_CACHE


def make_in_maps(x, w_packed, w_scale, w_bias):
    xtb, xt8 = host_prep_x(np.asarray(x, dtype=np.float32))
    wns, w8s, sts, bts = host_prep_w(np.asarray(w_packed),
                                     np.asarray(w_scale), np.asarray(w_bias))
    return [{"xt": xtb, "x8": xt8, "wn": wns[c], "w8": w8s[c],
             "st": sts[c], "bt": bts[c]}
            for c in range(N_CORES)]


def assemble_out(results):
    yt = np.concatenate([np.asarray(r["yt"]) for r in results], axis=0)
    return np.ascontiguousarray(yt.T).reshape(B, S, OUT_F).astype(np.float32)


def run(x, w_packed, w_scale, w_bias, trace=False, **kw):
    nc = get_nc()
    in_maps = make_in_maps(x, w_packed, w_scale, w_bias)
    res = bass_utils.run_bass_kernel_spmd(
        nc, in_maps, core_ids=list(range(N_CORES)), trace=trace, **kw)
    return assemble_out(res.results), res


def kernel(x, w_packed, w_scale, w_bias):
    out, _ = run(x, w_packed, w_scale, w_bias, trace=False)
    return out


# revision 26
# speedup vs baseline: 1.0128x; 1.0128x over previous
"""GroupQuantLinear int4 dequant + linear on 8 Trainium2 NeuronCores.

y = x @ W^T,  W = dequant(w_packed)*w_scale + w_bias  (group size 64)

Column-parallel sharding: 1536 output rows per core, x replicated.
Per core the contraction K=8192 is split into 64 k-tiles of 128
partitions (partition == quant group, k == position in group), plus one
extra k-tile that multiplies per-group x sums against the bias rows so
the bias term rides the same PSUM accumulation. 12 o-tiles of 128 rows
run as 2 passes of 6 PSUM banks; out [128 o, 512 t] fp32 per bank.

Performance structure (measured on HW, perfetto-profiled):
  - Mixed-precision contraction: the first 50 k-tiles are bf16
    (u8 nibbles streamed from HBM, dequantized on the DVE with a single
    u8*bf16 multiply per k-tile); the last NF=14 k-tiles are fp8e4
    DoubleRow pairs (2 contraction slices per matmul = 2x PE rate),
    host-dequantized so they cost the same HBM bytes as nibbles and no
    DVE work. fp8 error scales as sqrt(NF/64)*3.7%; measured rel err
    1.75e-2 against the fp32 reference (gate 2e-2).
  - ~30 dummy matmuls on a zeroed tile right after the framework
    preamble keep the PE busy from t~7.5us so the HAM clock-gate is at
    8/8 (2.4 GHz) before the real matmuls start, and bias matmuls are
    ordered first so the PE never goes idle (idle >3.4us re-throttles
    to 1.2 GHz).
  - The early DMA window is latency-critical and bandwidth-starved
    (~150 GB/s effective): only the opening tensors (bias/scale halves,
    xsum, first x and nibble chunks) are issued up front; all other
    input streams (remaining x chunks, second bias/scale halves, fp8 x
    and weights) are released inside the loop, gated on dequant
    progress via tiny gate-copies, so the SDMA engines never round-rob
    the critical path against bulk traffic.
  - Weight nibbles stream on the sync HWDGE queue in ramped chunks with
    a small split head per chunk; x streams on the scalar HWDGE queue;
    outputs leave as bf16 (halves output bytes; host converts to f32).
  - PSUM evacuation: pass-0 on scalar copies + gpsimd DMAs mid-kernel;
    the final pass alternates vector/scalar copies and gpsimd/sync DMA
    queues to shorten the post-last-matmul tail.
"""
import os
import sys

for _p in ("/opt/trn_rl_repo",):
    if _p not in sys.path and os.path.isdir(_p):
        sys.path.insert(0, _p)

import numpy as np
import ml_dtypes

import concourse.bacc as bacc
import concourse.mybir as mybir
import concourse.tile as tile
from concourse import bass_utils

# ---- problem constants (hardcoded per contract) ----
B, S, IN_F, OUT_F = 4, 128, 8192, 12288
GS = 64                 # quant group size
NG = IN_F // GS         # 128 groups == partitions per k-tile
N_CORES = 8
O_CORE = OUT_F // N_CORES   # 1536
T = B * S                   # 512 tokens
N_OPASS = 2                 # PSUM-capacity passes over output tiles
N_WARM = 30                 # HAM-warmup dummy matmuls
NF = 14                     # trailing k-tiles computed in fp8 DoubleRow
NB16 = GS - NF              # leading bf16 k-tiles
NPAIR = NF // 2
NKB = 1 + NB16              # xsum + bf16 k-tiles in the bf16 x tensor

F8 = ml_dtypes.float8_e4m3  # TRN float8e4: e4m3, bias 7, max +-240


def host_prep_x(x):
    """x [B,S,I] fp32 -> (xtb [128, NKB, T] bf16, xt8 [128, NPAIR, 2, T] fp8).

    xtb[:,0] is the per-group x sum (bias k-tile); xtb[:,1+k] is group
    position k for k < NB16. xt8[:,p,i] is group position NB16 + 2p + i.
    """
    x2 = x.reshape(T, NG, GS)
    xtb = np.empty((NG, NKB, T), dtype=np.float32)
    xtb[:, 0] = x2.sum(axis=2, dtype=np.float64).T
    xtb[:, 1:] = x2.transpose(1, 2, 0)[:, :NB16]
    xt8 = np.ascontiguousarray(
        x2.transpose(1, 2, 0)[:, NB16:].reshape(NG, NPAIR, 2, T))
    return xtb.astype(ml_dtypes.bfloat16), xt8.astype(F8)


def host_prep_w(w_packed, w_scale, w_bias):
    """-> per-core (wn u8 nibbles for the bf16 k-tiles, w8 fp8 dequantized
    weights for the trailing NF k-tiles (paired for DoubleRow), sT bf16,
    bT bf16). The fp8 section is dequantized on the host: same bytes as
    nibbles in HBM, but zero on-chip DVE work."""
    p4 = w_packed.reshape(OUT_F, NG, 4, 4)
    nibs = np.stack([(p4 >> (4 * i)) & 0xF for i in range(4)], axis=-2)
    u = nibs.reshape(OUT_F, NG, GS).astype(np.uint8)        # [O, G, 64]
    OH = O_CORE // N_OPASS
    wns, w8s, sts, bts = [], [], [], []
    for c in range(N_CORES):
        sl = slice(c * O_CORE, (c + 1) * O_CORE)
        uc = u[sl].transpose(1, 2, 0)                        # [128, 64, Oc]
        st = np.ascontiguousarray(w_scale[sl, :, 0].T).astype(ml_dtypes.bfloat16)
        wn = np.empty((N_OPASS, NG, NB16, OH), dtype=np.uint8)
        w8 = np.empty((N_OPASS, NG, NPAIR, 2, OH), dtype=F8)
        wq = (uc[:, NB16:].astype(np.float32)
              * st.astype(np.float32)[:, None, :])           # [128, NF, Oc]
        for p in range(N_OPASS):
            wn[p] = uc[:, :NB16, p * OH:(p + 1) * OH]
            w8[p] = wq[:, :, p * OH:(p + 1) * OH].reshape(
                NG, NPAIR, 2, OH).astype(F8)
        wns.append(wn)
        w8s.append(w8)
        sts.append(st)
        bts.append(np.ascontiguousarray(w_bias[sl, :, 0].T)
                   .astype(ml_dtypes.bfloat16))
    return wns, w8s, sts, bts


def build():
    """Build the per-core bass program (identical on all cores)."""
    NOJ = O_CORE // 128
    OPP = NOJ // N_OPASS
    OH = OPP * 128

    WCH0 = [1, 3, 4] + [10] * 4 + [2]     # pass-0 nibble chunks (sum NB16)
    WCH1 = [10] * 5                       # pass-1 nibble chunks (sum NB16)
    assert sum(WCH0) == NB16 and sum(WCH1) == NB16
    # bf16 x chunks. The first N_OPEN_X k-tiles ride the sync queue right
    # after the first weight chunk (in-queue order keeps them behind the
    # latency-critical tensors); the rest are released inside the pass-0
    # loop, gated on dequant progress, so the SDMA engines are not flooded
    # with x traffic while the opening tensors and nibble stream flow.
    N_OPEN_X = 4
    XCH_OPEN = [1, 1, 2]                  # upfront scalar chunks (sum N_OPEN_X)
    XCH = [2, 2] + [4] * 10 + [2]         # gated chunks (sum NB16 - N_OPEN_X)
    assert sum(XCH_OPEN) == N_OPEN_X and sum(XCH) == NB16 - N_OPEN_X

    nc = bacc.Bacc("TRN2", target_bir_lowering=False)
    xt_d = nc.dram_tensor("xt", [NG, NKB, T], mybir.dt.bfloat16,
                          kind="ExternalInput")
    x8_d = nc.dram_tensor("x8", [NG, NPAIR, 2, T], mybir.dt.float8e4,
                          kind="ExternalInput")
    wn_d = nc.dram_tensor("wn", [N_OPASS, NG, NB16, OH], mybir.dt.uint8,
                          kind="ExternalInput")
    w8_d = nc.dram_tensor("w8", [N_OPASS, NG, NPAIR, 2, OH],
                          mybir.dt.float8e4, kind="ExternalInput")
    st_d = nc.dram_tensor("st", [NG, O_CORE], mybir.dt.bfloat16, kind="ExternalInput")
    bt_d = nc.dram_tensor("bt", [NG, O_CORE], mybir.dt.bfloat16, kind="ExternalInput")
    yt_d = nc.dram_tensor("yt", [O_CORE, T], mybir.dt.bfloat16,
                          kind="ExternalOutput")

    with tile.TileContext(nc) as tc:
        with (
            tc.tile_pool(name="resident", bufs=1) as rpool,
            tc.tile_pool(name="nibs", bufs=3) as bpool,
            tc.tile_pool(name="wts", bufs=12) as wpool,
            tc.tile_pool(name="wts8", bufs=2) as w8pool,
            tc.tile_pool(name="evac", bufs=6) as opool,
            tc.tile_pool(name="psum", bufs=8, space="PSUM") as ppool,
        ):
            # --- PE prewarm: dummy matmuls on a zeroed tile so the HAM
            # clock-gate is already 8/8 when the first real matmul issues.
            warm_w = rpool.tile([128, 128], mybir.dt.bfloat16)
            nc.vector.memset(warm_w[:], 0)
            warm_ps = ppool.tile([128, T], mybir.dt.float32, tag="ps",
                                 name="warm")
            for _ in range(N_WARM):
                nc.tensor.matmul(warm_ps[:, :128], warm_w[:], warm_w[:],
                                 start=True, stop=True, skip_group_check=True)

            # --- opening DMAs, one stream per queue ---
            st_s = rpool.tile([NG, O_CORE], mybir.dt.bfloat16)
            bt_s = rpool.tile([NG, O_CORE], mybir.dt.bfloat16)
            x8_s = rpool.tile([NG, NPAIR, 2, T], mybir.dt.float8e4)
            xt_s = rpool.tile([NG, NKB, T], mybir.dt.bfloat16)
            nc.scalar.dma_start(bt_s[:, :OH], bt_d[:, :OH])
            nc.scalar.dma_start(st_s[:, :OH], st_d[:, :OH])
            nc.sync.dma_start(xt_s[:, 0:1, :], xt_d[:, 0:1, :])
            gate_s = rpool.tile([1, 2], mybir.dt.bfloat16)
            k0 = 1
            for ch in XCH_OPEN:
                nc.scalar.dma_start(xt_s[:, k0:k0 + ch, :],
                                    xt_d[:, k0:k0 + ch, :])
                k0 += ch
            # map: dequant k -> x chunks to release right after it (10 k-tile
            # lead over the first matmul that consumes the chunk)
            release_at = {}
            kx = N_OPEN_X                     # first k-tile of next chunk
            for ch in XCH:
                release_at.setdefault(max(0, kx - 10), []).append((kx + 1, ch))
                kx += ch
            X8_GATE_K = 28                    # release x8 after this dequant

            for p in range(N_OPASS):
                oo = p * OH
                psums = [ppool.tile([128, T], mybir.dt.float32, tag="ps",
                                    name=f"ps_{p}_{j}")
                         for j in range(OPP)]
                # bias k-tile first: needs only xsum (xt idx 0) + bt
                for j in range(OPP):
                    nc.tensor.matmul(
                        psums[j][:],
                        bt_s[:, oo + j * 128: oo + (j + 1) * 128],
                        xt_s[:, 0, :],
                        start=True, stop=False)
                k0 = 0
                for ch in (WCH0 if p == 0 else WCH1):
                    nt = bpool.tile([NG, ch, OH], mybir.dt.uint8, tag="nib",
                                    name=f"nib_{p}_{k0}")
                    if ch >= 4:
                        # split the chunk DMA so the first dequants can
                        # start before the whole chunk has landed
                        nc.sync.dma_start(nt[:, :2, :],
                                          wn_d[p, :, k0:k0 + 2, :])
                        nc.sync.dma_start(nt[:, 2:, :],
                                          wn_d[p, :, k0 + 2:k0 + ch, :])
                    else:
                        nc.sync.dma_start(nt[:], wn_d[p, :, k0:k0 + ch, :])
                    for kk in range(ch):
                        k = k0 + kk
                        if True:
                            wt = wpool.tile([NG, OH], mybir.dt.bfloat16,
                                            tag="wt")
                            nc.vector.tensor_mul(wt[:], nt[:, kk, :],
                                                 st_s[:, oo:oo + OH])
                            if p == 0:
                                # release paced x/x8 DMAs tied to dequant
                                # progress via tiny gate copies (FIFO on the
                                # issuing engine orders the dma after them)
                                for (xs, ch2) in release_at.get(k, ()):
                                    nc.scalar.copy(gate_s[:], wt[:1, :2])
                                    nc.scalar.dma_start(
                                        xt_s[:, xs:xs + ch2, :],
                                        xt_d[:, xs:xs + ch2, :])
                                if k == 16:
                                    nc.scalar.copy(gate_s[:], wt[:1, :2])
                                    nc.scalar.dma_start(bt_s[:, OH:],
                                                        bt_d[:, OH:])
                                elif k == 20:
                                    nc.scalar.copy(gate_s[:], wt[:1, :2])
                                    nc.scalar.dma_start(st_s[:, OH:],
                                                        st_d[:, OH:])
                                elif k == X8_GATE_K:
                                    nc.scalar.copy(gate_s[:], wt[:1, :2])
                                    nc.scalar.dma_start(x8_s[:], x8_d[:])
                            for j in range(OPP):
                                nc.tensor.matmul(
                                    psums[j][:],
                                    wt[:, j * 128:(j + 1) * 128],
                                    xt_s[:, k + 1, :],
                                    start=False, stop=False)
                    k0 += ch
                # trailing NF k-tiles: host-dequantized fp8 DoubleRow pairs
                w8s = w8pool.tile([NG, NPAIR, 2, OH], mybir.dt.float8e4,
                                  tag="w8")
                nc.sync.dma_start(w8s[:, :2], w8_d[p, :, :2])
                nc.sync.dma_start(w8s[:, 2:], w8_d[p, :, 2:])
                for pr in range(NPAIR):
                    for j in range(OPP):
                        nc.tensor.matmul(
                            psums[j][:],
                            w8s[:, pr, :, j * 128:(j + 1) * 128],
                            x8_s[:, pr, :, :],
                            start=False, stop=(pr == NPAIR - 1),
                            perf_mode=mybir.MatmulPerfMode.DoubleRow)
                if p == 0:
                    # mid-kernel evacuation: scalar copies (ACT can read
                    # PSUM; it is idle here), DMAs on the idle gpsimd queue
                    for j in range(OPP):
                        ot = opool.tile([128, T], mybir.dt.bfloat16, tag="ot")
                        nc.scalar.copy(ot[:], psums[j][:])
                        nc.gpsimd.dma_start(
                            yt_d[oo + j * 128: oo + (j + 1) * 128, :], ot[:])
                else:
                    # final evacuation: alternate vector/scalar copies and
                    # gpsimd/sync DMA queues so the post-last-matmul tail
                    # is short (gpsimd cannot read PSUM, so no copies there)
                    for j in range(OPP):
                        rows = slice(oo + j * 128, oo + (j + 1) * 128)
                        if j == OPP - 1:
                            # split the last tile: both halves leave on the
                            # fast HWDGE queues right after the last matmul
                            o0 = opool.tile([128, T // 2], mybir.dt.bfloat16,
                                            tag="ot")
                            nc.vector.tensor_copy(o0[:], psums[j][:, :T // 2])
                            nc.sync.dma_start(yt_d[rows, :T // 2], o0[:])
                            o1 = opool.tile([128, T // 2], mybir.dt.bfloat16,
                                            tag="ot")
                            nc.scalar.copy(o1[:], psums[j][:, T // 2:])
                            nc.scalar.dma_start(yt_d[rows, T // 2:], o1[:])
                        elif j % 2 == 0:
                            ot = opool.tile([128, T], mybir.dt.bfloat16,
                                            tag="ot")
                            nc.vector.tensor_copy(ot[:], psums[j][:])
                            nc.gpsimd.dma_start(yt_d[rows, :], ot[:])
                        else:
                            ot = opool.tile([128, T], mybir.dt.bfloat16,
                                            tag="ot")
                            nc.scalar.copy(ot[:], psums[j][:])
                            nc.sync.dma_start(yt_d[rows, :], ot[:])

    nc.compile()
    return nc


_NC_CACHE = None


def get_nc():
    global _NC_CACHE
    if _NC_CACHE is None:
        _NC_CACHE = build()
    return _NC_CACHE


def make_in_maps(x, w_packed, w_scale, w_bias):
    xtb, xt8 = host_prep_x(np.asarray(x, dtype=np.float32))
    wns, w8s, sts, bts = host_prep_w(np.asarray(w_packed),
                                     np.asarray(w_scale), np.asarray(w_bias))
    return [{"xt": xtb, "x8": xt8, "wn": wns[c], "w8": w8s[c],
             "st": sts[c], "bt": bts[c]}
            for c in range(N_CORES)]


def assemble_out(results):
    yt = np.concatenate([np.asarray(r["yt"]) for r in results], axis=0)
    return np.ascontiguousarray(yt.T).reshape(B, S, OUT_F).astype(np.float32)


def run(x, w_packed, w_scale, w_bias, trace=False, **kw):
    nc = get_nc()
    in_maps = make_in_maps(x, w_packed, w_scale, w_bias)
    res = bass_utils.run_bass_kernel_spmd(
        nc, in_maps, core_ids=list(range(N_CORES)), trace=trace, **kw)
    return assemble_out(res.results), res


def kernel(x, w_packed, w_scale, w_bias):
    out, _ = run(x, w_packed, w_scale, w_bias, trace=False)
    return out
